# revision 43
# baseline (speedup 1.0000x reference)
"""Trainium2 Bass kernel for nn_AggregationMPNN (gated-attention MPNN + GRU).

Data-parallel over the batch: 64 graphs per core on 8 NeuronCores.  The
~19%-dense adjacency is exploited by processing only real (i,j) edges:
the host pairs graphs (sorted pairing to minimize padding), packs each
pair's directed edges into one padded stream (E2 columns), and builds
one-hot selection matrices so every gather / scatter / mask / softmax
reduction becomes a TensorE matmul:

  - lhsT column e of `edgesA` holds [onehot(j_e) ; edge_feat_e] for the
    owning pair member; one K=128 matmul against [np_j ; We] computes
    tanh-input = edge_proj + nghb_proj for 128 edges at once, and the
    same lhsT against [emb_j ; 0] gathers emb_{j_e}.
  - att-denominator and message sums scatter per node i via selI
    one-hots (isolated nodes get a permanently-padded slot with u=1,
    msg=0; their h drifts but is provably unused: adjacency is
    symmetric and the readout masks them).
  - softmax needs no max-subtraction: tanh output is in [-1,1].

ScalarE does tanh/exp/sigmoid, VectorE cheap elementwise only.  The
attention stack runs in fp16 (1 cyc/row on PE, ~1e-3 rounding), GRU
matmuls in fp16 against an fp32 master h kept transposed
[H=128, (graph,node)] in SBUF for all 3 passes.  All DMA transfers are
host-pre-laid-out to be fully contiguous per partition.  The readout
folds the node mask and the graph-sum into a final selG matmul that
also undoes the host-side graph permutation.
"""

import os
import sys
from contextlib import ExitStack

import numpy as np

for _p in ("/root/.axon_site/_ro/trn_rl_repo", "/opt/trn_rl_repo"):
    if _p not in sys.path and os.path.isdir(_p):
        sys.path.append(_p)

import concourse.bacc as bacc  # noqa: E402
import concourse.mybir as mybir  # noqa: E402
import concourse.tile as tile  # noqa: E402
from concourse.bass_utils import run_bass_kernel_spmd  # noqa: E402

N = 40          # nodes per graph
H = 128         # hidden dim
M = 128         # message dim
FE = 16         # edge feature dim
AUG = N + FE    # augmented edge feature dim (selJ one-hot ++ features)
OUT = 128       # readout dim
PASSES = int(os.environ.get("K_PASSES", "3"))
NCORES = 8

f32 = mybir.dt.float32
f32r = mybir.dt.float32r
f16 = mybir.dt.float16
AF = mybir.ActivationFunctionType
ALU = mybir.AluOpType
NP16 = mybir.dt.np(f16)


# ---------------------------------------------------------------- host prep

def _host_prep(nodes, edges):
    """Edge-list + selection-matrix prep for all graphs (fp16 on the wire)."""
    B = nodes.shape[0]
    adj = edges.sum(axis=3) > 0                      # (B, N, N)  "j is nbr of i"
    ne = adj.reshape(B, -1).sum(axis=1)              # directed edge counts
    E = int(max(128, int(-(-(int(ne.max()) + 1) // 128)) * 128))

    b_idx, i_idx, j_idx = np.nonzero(adj)
    offs = np.zeros(B + 1, dtype=np.int64)
    np.cumsum(ne, out=offs[1:])
    pos = np.arange(len(b_idx)) - offs[b_idx]

    # rows 0:N = one-hot of j (doubles as selJ), rows N:AUG = edge features
    edgesA = np.zeros((B, AUG, E), dtype=NP16)
    edgesA[b_idx, j_idx, pos] = 1.0
    edgesA[b_idx[:, None], N + np.arange(FE)[None, :], pos[:, None]] = \
        edges[b_idx, i_idx, j_idx, :].astype(NP16)

    selI = np.zeros((B, E, N), dtype=NP16)
    selI[b_idx, pos, i_idx] = 1.0
    node_mask = adj.any(axis=2)                      # (B, N)
    iso_b, iso_i = np.nonzero(~node_mask)
    # slot E-1 is never a real edge (E >= max_ne + 1): u[E-1] = exp(tanh(0)) = 1
    selI[iso_b, E - 1, iso_i] = 1.0

    return {
        "edgesA": edgesA,
        "selI": selI,
        "node_mask": node_mask.astype(np.float32),
        "E": E,
    }


# ------------------------------------------------------------- bass builder

def _build_nc(G, E):
    """One SPMD NeuronCore program processing G graphs with edge capacity E."""
    EC = E // 128            # 128-edge chunks per graph
    GN = G * N               # columns of the transposed node layout
    RCH = GN // 128          # readout row-chunks
    assert GN % 128 == 0 and GN % 512 == 0

    nc = bacc.Bacc("TRN2", target_bir_lowering=False, debug=False,
                   num_devices=NCORES)

    dp = nc.declare_dram_parameter
    edgesA_d = dp("edgesA", [G // 8, AUG, 8 * E], f16, isOutput=False)
    selI_d = dp("selI", [G // 8, 128, 8 * EC * N], f16, isOutput=False)
    nodesT_d = dp("nodesT", [128, GN], f32, isOutput=False)
    selG_d = dp("selG", [128, RCH * G], f32, isOutput=False)
    WeG_d = dp("WeG2", [FE, G * 256], f16, isOutput=False)  # [We | 0] per graph
    Wn_d = dp("Wn16", [H, M], f16, isOutput=False)
    Wm_d = dp("Wm16", [H, M], f16, isOutput=False)
    Wi_d = dp("Wi16", [M, 3 * H], f16, isOutput=False)
    Wh_d = dp("Wh16", [H, 3 * H], f16, isOutput=False)
    I128_d = dp("I128", [128, 128], f16, isOutput=False)
    brz_d = dp("brz", [128, 2], f32, isOutput=False)       # bi+bh for r,z gates
    bin_d = dp("bin", [128, 1], f32, isOutput=False)       # bi n-gate
    bhn_d = dp("bhn", [1, 128], f16, isOutput=False)       # bh n-gate
    Wg_top_d = dp("Wg_top", [H, OUT], f32, isOutput=False)
    Wg_bot_d = dp("Wg_bot", [H, OUT], f32, isOutput=False)
    Wo_top_d = dp("Wo_top", [H, OUT], f32, isOutput=False)
    Wo_bot_d = dp("Wo_bot", [H, OUT], f32, isOutput=False)
    bg_d = dp("bg", [1, OUT], f32, isOutput=False)
    bo_d = dp("bo", [1, OUT], f32, isOutput=False)
    out_d = dp("out", [G, OUT], f32, isOutput=True)

    GB = 8                          # graphs per DMA load group
    SB = 4                          # graphs per np/emb psum staging group
    PAIR = max(1, 1536 // (EC * 256))  # graphs per e-psum group
    NCH = PAIR * EC                 # chunks per e-psum group
    CL = NCH * 128                  # ep columns per e-psum group
    EXPG = 1                        # e-psum groups per exp batch
    n_gru_chunks = GN // 512
    assert GB % (PAIR * EXPG) == 0

    with tile.TileContext(nc) as tc, ExitStack() as ctx:
        const = ctx.enter_context(tc.tile_pool(name="const", bufs=1))
        state = ctx.enter_context(tc.tile_pool(name="state", bufs=1))
        ld = ctx.enter_context(tc.tile_pool(name="ld", bufs=4))
        work = ctx.enter_context(tc.tile_pool(name="work", bufs=3))
        gw = ctx.enter_context(tc.tile_pool(name="gw", bufs=2))
        psA = ctx.enter_context(tc.tile_pool(name="psA", bufs=2, space="PSUM"))
        psB = ctx.enter_context(tc.tile_pool(name="psB", bufs=1, space="PSUM"))

        # ---- constants / weights (critical-path loads first: nodesT feeds
        # h/h16, Wn/Wm feed the first projection matmuls)
        def cload(shape, dt_, src, tag):
            t = const.tile(shape, dt_, tag=tag)
            nc.sync.dma_start(out=t[:], in_=src[:])
            return t

        nodesT = state.tile([128, GN], f32, tag="nodesT")
        nc.sync.dma_start(out=nodesT[:], in_=nodesT_d[:])
        Wn_sb = cload([H, M], f16, Wn_d, "c_wn")
        Wm_sb = cload([H, M], f16, Wm_d, "c_wm")
        WeG_sb_dummy = None
        feat_all = state.tile([128, P2 * 128], f16, tag="feat_all")
        nc.sync.dma_start(out=feat_all[:], in_=WeG_d[:])
        Wi_sb = cload([M, 3 * H], f16, Wi_d, "c_wi")
        Wh_sb = cload([H, 3 * H], f16, Wh_d, "c_wh")
        I128_sb = cload([128, 128], f16, I128_d, "c_i128")
        brz_sb = cload([128, 2], f32, brz_d, "c_brz")
        bin_sb = cload([128, 1], f32, bin_d, "c_bin")
        bhn_sb = cload([1, 128], f16, bhn_d, "c_bhn")
        Wg_top_sb = cload([H, OUT], f32, Wg_top_d, "c_wgt")
        Wg_bot_sb = cload([H, OUT], f32, Wg_bot_d, "c_wgb")
        Wo_top_sb = cload([H, OUT], f32, Wo_top_d, "c_wot")
        Wo_bot_sb = cload([H, OUT], f32, Wo_bot_d, "c_wob")
        bg_sb = cload([1, OUT], f32, bg_d, "c_bg")
        bo_sb = cload([1, OUT], f32, bo_d, "c_bo")
        selG_sb = const.tile([128, RCH * G], f32)
        nc.sync.dma_start(out=selG_sb[:], in_=selG_d[:])
        ones_sb = const.tile([1, 512], f16)
        nc.vector.memset(ones_sb[:], 1.0)
        ones32_sb = const.tile([1, 128], f32)
        nc.vector.memset(ones32_sb[:], 1.0)

        hT = state.tile([128, GN], f32, tag="hT")
        nc.vector.tensor_copy(out=hT[:], in_=nodesT[:])
        emb_all = state.tile([128, P2 * 128], f16, tag="emb_all")
        nc.vector.memset(emb_all[:], 0.0)

        h16 = state.tile([128, GN], f16, tag="h16")
        nc.vector.tensor_copy(out=h16[:], in_=nodesT[:])
        for p in range(PASSES):
            msgsT = state.tile([128, GN], f16, tag="msgsT")
            recipT = state.tile([128, GN], f32, tag="recipT")

            # per-graph projections: np_j = h_g Wn, emb_j = h_g Wm  [N, M]
            for s0 in range(0, G, SB):
                np_ps = psB.tile([N, SB * 128], f32, tag="acc0")
                emb_ps = psB.tile([N, SB * 128], f32, tag="acc1")
                for k in range(SB):
                    g = s0 + k
                    hg = h16[:, g * N:(g + 1) * N]
                    nc.tensor.matmul(np_ps[:, k * 128:(k + 1) * 128],
                                     hg, Wn_sb[:], start=True, stop=True)
                    nc.tensor.matmul(emb_ps[:, k * 128:(k + 1) * 128],
                                     hg, Wm_sb[:], start=True, stop=True)
                dst = feat_all[0:N, s0 * 256:(s0 + SB) * 256].rearrange(
                    "p (g two m) -> p g two m", two=2, m=128)
                nc.scalar.copy(out=dst[:, :, 0, :], in_=np_ps[:].rearrange(
                    "p (g m) -> p g m", g=SB))
                nc.vector.tensor_copy(out=dst[:, :, 1, :], in_=emb_ps[:].rearrange(
                    "p (g m) -> p g m", g=SB))

            # attention + message aggregation, edge-chunked
            for l0 in range(0, G, GB):          # DMA load group
                edgesA_sb = ld.tile([128, PGB * E], f16, tag="edgesA")
                nc.sync.dma_start(out=edgesA_sb[:], in_=edgesA_d[l0 // GB])
                selI_sb = ld.tile([128, PGB * EC * 2 * N], f16, tag="selI")
                nc.sync.dma_start(out=selI_sb[:], in_=selI_d[l0 // GB])

                den_ps = psB.tile([128, GB * N], f32, tag="acc0")
                msg_ps = psB.tile([128, GB * N], f32, tag="acc1")
                for x0 in range(0, GB, PAIR * EXPG):    # one exp batch
                    t_all = work.tile([128, EXPG * CL], f32, tag="t_all")
                    e_pss = []
                    for xe in range(EXPG):
                        p0 = x0 + xe * PAIR
                        e_ps = psA.tile([128, NCH * 256], f32, tag="e_ps")
                        e_pss.append(e_ps)
                        for q in range(NCH):
                            lg, c = p0 + q // EC, q % EC
                            g = l0 + lg
                            eA = edgesA_sb[:, lg * E + c * 128:lg * E + (c + 1) * 128]
                            nc.tensor.matmul(e_ps[:, q * 256:(q + 1) * 256],
                                             eA,
                                             feat_all[:, g * 256:(g + 1) * 256],
                                             start=True, stop=True)
                        ep_half = e_ps[:].rearrange(
                            "p (q two m) -> p q two m", two=2, m=128)[:, :, 0, :]
                        nc.scalar.activation(out=t_all[:, xe * CL:(xe + 1) * CL]
                                             .rearrange("p (q m) -> p q m", q=NCH),
                                             in_=ep_half, func=AF.Tanh)
                    u_all = work.tile([128, EXPG * CL], f16, tag="u_all")
                    nc.scalar.activation(out=u_all[:], in_=t_all[:], func=AF.Exp)
                    for xe in range(EXPG):
                        p0 = x0 + xe * PAIR
                        uoff = xe * CL
                        embe_half = e_pss[xe][:].rearrange(
                            "p (q two m) -> p q two m", two=2, m=128)[:, :, 1, :]
                        w_sb = work.tile([128, CL], f16, tag="w_sb")
                        nc.vector.tensor_mul(
                            w_sb[:].rearrange("p (q m) -> p q m", q=NCH),
                            u_all[:, uoff:uoff + CL]
                            .rearrange("p (q m) -> p q m", q=NCH),
                            embe_half)
                        for q in range(NCH):
                            lg, c = p0 + q // EC, q % EC
                            sI = selI_sb[:, (lg * EC + c) * N:(lg * EC + c + 1) * N]
                            gcols = slice(lg * N, (lg + 1) * N)
                            uq = slice(uoff + q * 128, uoff + (q + 1) * 128)
                            wq = slice(q * 128, (q + 1) * 128)
                            nc.tensor.matmul(den_ps[:, gcols], u_all[:, uq], sI,
                                             start=(c == 0), stop=(c == EC - 1),
                                             skip_group_check=True)
                            nc.tensor.matmul(msg_ps[:, gcols], w_sb[:, wq], sI,
                                             start=(c == 0), stop=(c == EC - 1),
                                             skip_group_check=True)
                # normalize this group's messages straight out of PSUM so the
                # GRU can start before the last load group finishes
                gstart = l0 * N
                rslc = slice(gstart, gstart + GB * N)
                nc.vector.reciprocal(out=recipT[:, rslc], in_=den_ps[:])
                nc.vector.tensor_mul(msgsT[:, rslc], msg_ps[:],
                                     recipT[:, rslc])

            # GRU update (transposed layout), h <- (1-z)*n + z*h
            for q in range(n_gru_chunks):
                S = slice(q * 512, (q + 1) * 512)
                mS = msgsT[:, S]
                hS = h16[:, S]
                r_ps = psA.tile([128, 512], f32, tag="e_ps")
                nc.tensor.matmul(r_ps[:], Wi_sb[:, 0:128], mS,
                                 start=True, stop=False)
                nc.tensor.matmul(r_ps[:], Wh_sb[:, 0:128], hS,
                                 start=False, stop=True)
                r_sb = gw.tile([128, 512], f32, tag="r_sb")
                nc.scalar.activation(out=r_sb[:], in_=r_ps[:], func=AF.Tanh,
                                     bias=brz_sb[:, 0:1], scale=0.5)
                z_ps = psA.tile([128, 512], f32, tag="e_ps")
                nc.tensor.matmul(z_ps[:], Wi_sb[:, 128:256], mS,
                                 start=True, stop=False)
                nc.tensor.matmul(z_ps[:], Wh_sb[:, 128:256], hS,
                                 start=False, stop=True)
                z_sb = gw.tile([128, 512], f32, tag="z_sb")
                nc.scalar.activation(out=z_sb[:], in_=z_ps[:], func=AF.Tanh,
                                     bias=brz_sb[:, 1:2], scale=0.5)
                ghn_ps = psA.tile([128, 512], f32, tag="e_ps")
                nc.tensor.matmul(ghn_ps[:], Wh_sb[:, 256:384], hS,
                                 start=True, stop=False)
                nc.tensor.matmul(ghn_ps[:], bhn_sb[:], ones_sb[:],
                                 start=False, stop=True)
                gin_ps = psA.tile([128, 512], f32, tag="e_ps")
                nc.tensor.matmul(gin_ps[:], Wi_sb[:, 256:384], mS,
                                 start=True, stop=False)
                rgh_sb = gw.tile([128, 512], f16, tag="rgh_sb")
                nc.vector.scalar_tensor_tensor(rgh_sb[:], r_sb[:], 1.0, ghn_ps[:],
                                               op0=ALU.add, op1=ALU.mult)
                nc.tensor.matmul(gin_ps[:], I128_sb[:], rgh_sb[:],
                                 start=False, stop=True)
                n_sb = gw.tile([128, 512], f32, tag="n_sb")
                nc.scalar.activation(out=n_sb[:], in_=gin_ps[:], func=AF.Tanh,
                                     bias=bin_sb[:])
                d_sb = gw.tile([128, 512], f32, tag="d_sb")
                nc.vector.tensor_sub(d_sb[:], hT[:, S], n_sb[:])
                zd_sb = gw.tile([128, 512], f32, tag="zd_sb")
                nc.vector.scalar_tensor_tensor(zd_sb[:], z_sb[:], 1.0, d_sb[:],
                                               op0=ALU.add, op1=ALU.mult)
                nc.vector.scalar_tensor_tensor(hT[:, S], zd_sb[:], 0.5, n_sb[:],
                                               op0=ALU.mult, op1=ALU.add)
                nc.vector.tensor_copy(out=h16[:, S], in_=hT[:, S])

        # ---- gated readout
        out_ps = psB.tile([G, OUT], f32, tag="acc0")
        for q in range(RCH):
            R = slice(q * 128, (q + 1) * 128)
            gate_ps = psA.tile([128, OUT], f32, tag="e_ps")
            nc.tensor.matmul(gate_ps[:], hT[:, R], Wg_top_sb[:],
                             start=True, stop=False)
            nc.tensor.matmul(gate_ps[:], nodesT[:, R], Wg_bot_sb[:],
                             start=False, stop=False)
            nc.tensor.matmul(gate_ps[:], ones32_sb[:], bg_sb[:],
                             start=False, stop=True)
            gate_sb = work.tile([128, OUT], f32, tag="gate_sb")
            nc.scalar.activation(out=gate_sb[:], in_=gate_ps[:], func=AF.Tanh,
                                 scale=0.5)
            embo_ps = psA.tile([128, OUT], f32, tag="e_ps")
            nc.tensor.matmul(embo_ps[:], hT[:, R], Wo_top_sb[:],
                             start=True, stop=False)
            nc.tensor.matmul(embo_ps[:], nodesT[:, R], Wo_bot_sb[:],
                             start=False, stop=False)
            nc.tensor.matmul(embo_ps[:], ones32_sb[:], bo_sb[:],
                             start=False, stop=True)
            prod_sb = work.tile([128, OUT], f32, tag="prod_sb")
            nc.vector.scalar_tensor_tensor(prod_sb[:], gate_sb[:], 1.0, embo_ps[:],
                                           op0=ALU.add, op1=ALU.mult)
            nc.tensor.matmul(out_ps[:], selG_sb[:, q * G:(q + 1) * G], prod_sb[:],
                             start=(q == 0), stop=(q == RCH - 1))
        out_sb = work.tile([G, OUT], f32, tag="out_sb")
        nc.scalar.copy(out=out_sb[:], in_=out_ps[:])
        nc.sync.dma_start(out=out_d[:], in_=out_sb[:])

    nc.compile()
    return nc


_NC_CACHE = {}


def _get_nc(G, E):
    key = (G, E)
    if key not in _NC_CACHE:
        _NC_CACHE[key] = _build_nc(G, E)
    return _NC_CACHE[key]


def _weg2(We16, G):
    w = np.zeros((FE, G, 2, 128), dtype=NP16)
    w[:, :, 0, :] = We16[:, None, :]
    return np.ascontiguousarray(w.reshape(FE, G * 256))


def _weg128(We16, P2):
    w = np.zeros((128, 128), dtype=NP16)
    w[N:N + FE, :] = We16
    w[64 + N:64 + N + FE, :] = We16
    return np.ascontiguousarray(np.broadcast_to(
        w[:, None, :], (128, P2, 128)).reshape(128, P2 * 128))


# ------------------------------------------------------------------ driver

def kernel(nodes, edges, We, Wn, Wm, Wi, Wh, bi, bh, Wg, bg, Wo, bo):
    nodes = np.asarray(nodes, dtype=np.float32)
    edges = np.asarray(edges, dtype=np.float32)
    B = nodes.shape[0]
    assert B % NCORES == 0
    G = B // NCORES
    GN = G * N
    RCH = GN // 128

    prep = _host_prep(nodes, edges)
    E = prep["E"]

    bi = np.asarray(bi, dtype=np.float32)
    bh = np.asarray(bh, dtype=np.float32)
    Wg = np.asarray(Wg, dtype=np.float32)
    Wo = np.asarray(Wo, dtype=np.float32)
    We16 = np.asarray(We, dtype=np.float32).astype(NP16)
    shared = {
        "WeG2": _weg2(We16, G),
        "Wn16": np.asarray(Wn, dtype=np.float32).astype(NP16),
        "Wm16": np.asarray(Wm, dtype=np.float32).astype(NP16),
        "Wi16": np.ascontiguousarray(np.asarray(Wi, dtype=np.float32).astype(NP16)),
        "Wh16": np.ascontiguousarray(np.asarray(Wh, dtype=np.float32).astype(NP16)),
        "I128": (0.5 * np.eye(128)).astype(NP16),
        "brz": np.ascontiguousarray(
            0.5 * np.stack([bi[0:128] + bh[0:128], bi[128:256] + bh[128:256]],
                           axis=1)).astype(np.float32),
        "bin": np.ascontiguousarray(bi[256:384].reshape(128, 1)),
        "bhn": np.ascontiguousarray(bh[256:384].reshape(1, 128).astype(NP16)),
        "Wg_top": np.ascontiguousarray(Wg[:H]),
        "Wg_bot": np.ascontiguousarray(Wg[H:]),
        "Wo_top": np.ascontiguousarray(Wo[:H]),
        "Wo_bot": np.ascontiguousarray(Wo[H:]),
        "bg": np.ascontiguousarray(np.asarray(bg, dtype=np.float32).reshape(1, OUT)),
        "bo": np.ascontiguousarray(np.asarray(bo, dtype=np.float32).reshape(1, OUT)),
    }

    in_maps = []
    for c in range(NCORES):
        sl = slice(c * G, (c + 1) * G)
        nm = prep["node_mask"][sl]                       # (G, N)
        rows = nm.reshape(GN)
        colg = np.repeat(np.arange(G), N)
        selG = np.zeros((GN, G), dtype=np.float32)
        selG[np.arange(GN), colg] = rows
        EC = E // 128
        in_maps.append({
            **shared,
            "edgesA": np.ascontiguousarray(
                prep["edgesA"][sl].reshape(G // 8, 8, AUG, E)
                .transpose(0, 2, 1, 3).reshape(G // 8, AUG, 8 * E)),
            "selI": np.ascontiguousarray(
                prep["selI"][sl].reshape(G // 8, 8, EC, 128, N)
                .transpose(0, 3, 1, 2, 4).reshape(G // 8, 128, 8 * EC * N)),
            "nodesT": np.ascontiguousarray(nodes[sl].reshape(GN, H).T),
            "selG": np.ascontiguousarray(
                0.5 * selG.reshape(RCH, 128, G).transpose(1, 0, 2)
                .reshape(128, RCH * G)).astype(np.float32),
        })

    nc = _get_nc(G, E)
    res = run_bass_kernel_spmd(nc, in_maps, list(range(NCORES)))
    return np.concatenate([res.results[c]["out"] for c in range(NCORES)], axis=0)
